# revision 5
# baseline (speedup 1.0000x reference)
"""EG_GATLayer (multi-head edge-gated GAT) on 8 Trainium2 NeuronCores.

Strategy: sort edges by destination node on the host and assign each core a
contiguous range of 49 destination-node tiles (128 nodes each).  Each core
fully owns its output rows, so there are no collectives.  Node features h are
computed (replicated) on every core into a per-core ROTATED table (row r holds
node (r + core_base) mod 51200, achieved by rotating xT on the host), so
destination indices are local [0, 6272) and source indices split into two
int16-addressable half-tables for dma_gather.  Segment softmax + scatter-sum
are done with one-hot matmuls accumulating in PSUM; normalization by the
softmax denominator happens after aggregation (algebraically identical).
"""

import os
import sys

import numpy as np

for _p in ("/opt/trn_rl_repo", "/opt/pypackages"):
    if _p not in sys.path:
        sys.path.insert(0, _p)

import ml_dtypes

# ---- problem constants (hardcoded per contract) ----
N_NODES = 50000
N_EDGES = 800000
IN_DIM = 128
EDGE_DIM = 16
H = 8
D = 16
HD = H * D  # 128

P = 128
N_CORES = 8
NT_PER_CORE = 49
NT_G = NT_PER_CORE * N_CORES  # 392 tiles of 128 nodes >= 50000
NODES_PER_CORE = NT_PER_CORE * P  # 6272
H_BLOCKS = 25
H_ROWS = H_BLOCKS * 2048  # 51200 padded rows of h / cols of xT
H_HALF = H_ROWS // 2  # 25600 (int16-addressable half-table split)

NCT_LO_DEFAULT = 9
NCT_HI_DEFAULT = 9

# precision switches
MSG_BF16 = True  # one-hot + message/denominator accumulation matmuls in bf16
E_BF16 = True  # edge projection matmul in bf16

_last_results = None  # test.py introspection


def _wrap_idx(idx_list):
    """dma_gather index layout: [16, n/16] wrapped, replicated to 128 parts."""
    n = len(idx_list)
    assert n % 16 == 0
    w = idx_list.reshape(n // 16, 16).T  # [16, n/16]
    return np.tile(w, (8, 1)).astype(np.int16)


# --------------------------------------------------------------------------
# host-side preprocessing
# --------------------------------------------------------------------------
def _prep(x, edge_attr, W_node, W_edge, src, dst, nct_lo, nct_hi):
    f32 = np.float32
    bf16 = ml_dtypes.bfloat16
    nct = nct_lo + nct_hi
    src = np.ascontiguousarray(src).astype(np.int64)
    dst = np.ascontiguousarray(dst).astype(np.int64)

    order = np.argsort(dst, kind="stable")
    src_s = src[order]
    dst_s = dst[order]
    ea_s = np.ascontiguousarray(edge_attr[order]).astype(f32)

    tile_id = dst_s // P
    bounds = np.searchsorted(tile_id, np.arange(NT_G + 1))

    e_dt = bf16 if E_BF16 else f32
    # gather idx arrays, per tile: [klo+q (nct_lo+nct)*128 idxs | khi nct_hi*128]
    n_t0 = (nct_lo + nct) * P
    n_t1 = nct_hi * P
    idx0_all = np.zeros((N_CORES, NT_PER_CORE, P, n_t0 // 16), np.int16)
    idx1_all = np.zeros((N_CORES, NT_PER_CORE, P, n_t1 // 16), np.int16)
    dstloc_all = np.full((N_CORES, NT_PER_CORE, P, nct), -1.0, f32)
    eaT_all = np.zeros((N_CORES, NT_PER_CORE, 16, nct * P), e_dt)

    xT_pad = np.zeros((P, H_ROWS), f32)
    xT_pad[:, :N_NODES] = np.ascontiguousarray(x.astype(f32).T)

    in_maps = []
    xTs = []
    for c in range(N_CORES):
        base = c * NODES_PER_CORE
        xTs.append(np.ascontiguousarray(np.roll(xT_pad, -base, axis=1)))

    for t in range(NT_G):
        lo_b, hi_b = bounds[t], bounds[t + 1]
        cnt = hi_b - lo_b
        if cnt == 0:
            continue
        c = t // NT_PER_CORE
        j = t % NT_PER_CORE
        base = c * NODES_PER_CORE

        rot = (src_s[lo_b:hi_b] - base) % H_ROWS
        is_lo = rot < H_HALF
        n_lo = int(is_lo.sum())
        n_hi = cnt - n_lo
        assert n_lo <= nct_lo * P and n_hi <= nct_hi * P, (t, n_lo, n_hi)

        # slot assignment: lo edges fill chunks [0, nct_lo), hi fill the rest
        perm = np.concatenate([np.where(is_lo)[0], np.where(~is_lo)[0]])
        slot = np.empty(cnt, np.int64)
        slot[:n_lo] = np.arange(n_lo)
        slot[n_lo:] = nct_lo * P + np.arange(n_hi)
        rot_p = rot[perm]
        dst_p = dst_s[lo_b:hi_b][perm]
        ea_p = ea_s[lo_b:hi_b][perm]

        cc, pp = slot // P, slot % P
        dstloc_all[c, j, pp, cc] = (dst_p - t * P).astype(f32)
        eaT_all[c, j, :, cc * P + pp] = ea_p.astype(e_dt)

        # table-0 gather: k-lo (chunks 0..nct_lo) then q (all nct chunks)
        i0 = np.zeros(n_t0, np.int64)
        i0[slot[:n_lo]] = rot_p[:n_lo]
        qi = np.zeros(nct * P, np.int64)
        qi[slot] = dst_p - base
        i0[nct_lo * P:] = qi
        idx0_all[c, j] = _wrap_idx(i0)
        # table-1 gather: k-hi (chunks nct_lo..nct)
        i1 = np.zeros(n_t1, np.int64)
        i1[slot[n_lo:] - nct_lo * P] = rot_p[n_lo:] - H_HALF
        idx1_all[c, j] = _wrap_idx(i1)

    Wn_cat = np.ascontiguousarray(
        W_node.astype(f32).transpose(1, 0, 2).reshape(IN_DIM, HD)
    )
    We_cat = np.zeros((EDGE_DIM, HD + H), f32)
    We_cat[:, :HD] = W_edge.astype(f32).transpose(1, 0, 2).reshape(EDGE_DIM, HD)
    We_cat[:, HD:] = W_edge.astype(f32).sum(axis=2).T
    We_cat = We_cat.astype(e_dt)

    iota = np.broadcast_to(np.arange(P, dtype=f32), (P, P)).copy()

    for c in range(N_CORES):
        in_maps.append({
            "xT": xTs[c],
            "Wn": Wn_cat,
            "We": We_cat,
            "iota": iota,
            "idx0": np.ascontiguousarray(idx0_all[c]),
            "idx1": np.ascontiguousarray(idx1_all[c]),
            "dstloc": np.ascontiguousarray(dstloc_all[c]),
            "eaT": np.ascontiguousarray(eaT_all[c]),
        })
    return in_maps


# --------------------------------------------------------------------------
# device program
# --------------------------------------------------------------------------
def build_program(nct_lo, nct_hi):
    import concourse.bass as bass
    import concourse.mybir as mybir
    import concourse.tile as tile
    from concourse import bacc
    from contextlib import ExitStack

    f32 = mybir.dt.float32
    i16 = mybir.dt.int16
    bf16 = mybir.dt.bfloat16
    msg_dt = bf16 if MSG_BF16 else f32
    e_dt = bf16 if E_BF16 else f32
    nct = nct_lo + nct_hi
    n_t0 = (nct_lo + nct) * P
    n_t1 = nct_hi * P

    nc = bacc.Bacc("TRN2", target_bir_lowering=False, debug=False,
                   num_devices=N_CORES)

    xT_t = nc.dram_tensor("xT", [P, H_ROWS], f32, kind="ExternalInput")
    Wn_t = nc.dram_tensor("Wn", [IN_DIM, HD], f32, kind="ExternalInput")
    We_t = nc.dram_tensor("We", [EDGE_DIM, HD + H], e_dt, kind="ExternalInput")
    iota_t = nc.dram_tensor("iota", [P, P], f32, kind="ExternalInput")
    idx0_t = nc.dram_tensor("idx0", [NT_PER_CORE, P, n_t0 // 16], i16,
                            kind="ExternalInput")
    idx1_t = nc.dram_tensor("idx1", [NT_PER_CORE, P, n_t1 // 16], i16,
                            kind="ExternalInput")
    dstloc_t = nc.dram_tensor("dstloc", [NT_PER_CORE, P, nct], f32,
                              kind="ExternalInput")
    eaT_t = nc.dram_tensor("eaT", [NT_PER_CORE, 16, nct * P], e_dt,
                           kind="ExternalInput")
    out_t = nc.dram_tensor("out", [NODES_PER_CORE, HD], f32,
                           kind="ExternalOutput")
    h_t = nc.dram_tensor("h", [H_ROWS, HD], f32)  # internal scratch (rotated)

    def strided(ap, pattern, extra_off=0):
        return bass.AP(ap.tensor, ap.offset + extra_off, pattern)

    with tile.TileContext(nc) as tc, ExitStack() as ctx:
        const = ctx.enter_context(tc.tile_pool(name="const", bufs=1))
        xbp = ctx.enter_context(tc.tile_pool(name="xb", bufs=3))
        hbp = ctx.enter_context(tc.tile_pool(name="hb", bufs=3))
        hps = ctx.enter_context(tc.tile_pool(name="hps", bufs=2, space="PSUM"))
        idxp = ctx.enter_context(tc.tile_pool(name="idx", bufs=2))
        kqp = ctx.enter_context(tc.tile_pool(name="kq", bufs=2))
        ohp = ctx.enter_context(tc.tile_pool(name="oh", bufs=2))
        qkp = ctx.enter_context(tc.tile_pool(name="qk", bufs=2))
        smp = ctx.enter_context(tc.tile_pool(name="sm", bufs=2))
        gp = ctx.enter_context(tc.tile_pool(name="g", bufs=2))
        rhp = ctx.enter_context(tc.tile_pool(name="rh", bufs=2))
        outp = ctx.enter_context(tc.tile_pool(name="o", bufs=2))
        eps_pool = ctx.enter_context(tc.tile_pool(name="eps", bufs=4, space="PSUM"))
        accp = ctx.enter_context(tc.tile_pool(name="acc", bufs=2, space="PSUM"))

        Wn_sb = const.tile([IN_DIM, HD], f32)
        nc.sync.dma_start(Wn_sb[:], Wn_t.ap())
        We_sb = const.tile([EDGE_DIM, HD + H], e_dt)
        nc.sync.dma_start(We_sb[:], We_t.ap())
        iota_sb = const.tile([P, P], f32)
        nc.sync.dma_start(iota_sb[:], iota_t.ap())

        # ---- phase 0: h = x @ Wn for all nodes, row-major (rotated) ----
        for blk in range(H_BLOCKS):
            xb = xbp.tile([P, 2048], f32, tag="xb")
            nc.sync.dma_start(xb[:], xT_t.ap()[:, blk * 2048:(blk + 1) * 2048])
            hb = hbp.tile([P, 2048], f32, tag="hb")
            for g in range(4):
                hp = hps.tile([P, 512], f32, tag="hp")
                for s_ in range(4):
                    nc.tensor.matmul(
                        hp[:, s_ * P:(s_ + 1) * P],
                        lhsT=xb[:, (g * 4 + s_) * P:(g * 4 + s_ + 1) * P],
                        rhs=Wn_sb[:],
                        start=True, stop=True,
                    )
                nc.vector.tensor_copy(hb[:, g * 512:(g + 1) * 512], hp[:])
            # rows blk*2048 + (g*128 + p), iterate (p, g, f)
            h_out = strided(
                h_t.ap(), [[HD, P], [P * HD, 16], [1, HD]],
                extra_off=blk * 2048 * HD,
            )
            nc.sync.dma_start(h_out, hb[:].rearrange("p (g f) -> p g f", f=P))

        h_lo = h_t.ap()[:H_HALF, :]
        h_hi = h_t.ap()[H_HALF:, :]

        # ---- phase 1: per destination-node tile ----
        for t in range(NT_PER_CORE):
            idx0 = idxp.tile([P, n_t0 // 16], i16, tag="idx0")
            nc.sync.dma_start(idx0[:], idx0_t.ap()[t])
            idx1 = idxp.tile([P, n_t1 // 16], i16, tag="idx1")
            nc.sync.dma_start(idx1[:], idx1_t.ap()[t])
            dstloc = idxp.tile([P, nct], f32, tag="dstloc")
            nc.sync.dma_start(dstloc[:], dstloc_t.ap()[t])
            eaT = idxp.tile([16, nct * P], e_dt, tag="eaT")
            nc.sync.dma_start(eaT[:], eaT_t.ap()[t])

            # gathered rows, layout [k-lo (nct_lo) | q (nct) | k-hi (nct_hi)]
            kq = kqp.tile([P, (2 * nct) * P], f32, tag="kq")
            nc.gpsimd.dma_gather(
                out_ap=kq[:, :n_t0].rearrange("p (c f) -> p c f", f=P),
                in_ap=h_lo,
                idxs_ap=idx0[:],
                num_idxs=n_t0,
                num_idxs_reg=n_t0,
                elem_size=HD,
                single_packet=False,
            )
            nc.gpsimd.dma_gather(
                out_ap=kq[:, n_t0:].rearrange("p (c f) -> p c f", f=P),
                in_ap=h_hi,
                idxs_ap=idx1[:],
                num_idxs=n_t1,
                num_idxs_reg=n_t1,
                elem_size=HD,
                single_packet=False,
            )
            k_lo = kq[:, :nct_lo * P]
            qv = kq[:, nct_lo * P:(nct_lo + nct) * P]
            k_hi = kq[:, (nct_lo + nct) * P:]

            # one-hot [edge, local-node] (pad edges have dstloc=-1 -> all 0)
            onehot = ohp.tile([P, nct * P], msg_dt, tag="onehot")
            oh_ap = onehot[:].rearrange("p (c n) -> p c n", n=P)
            d_ap = strided(dstloc[:], [[dstloc[:].ap[0][0], P], [1, nct], [0, P]])
            i_ap = strided(iota_sb[:], [[iota_sb[:].ap[0][0], P], [0, nct], [1, P]])
            nc.vector.tensor_tensor(oh_ap, d_ap, i_ap, op=mybir.AluOpType.is_equal)

            # qk dot per head (k in two pieces)
            qk = qkp.tile([P, nct * P], f32, tag="qk")
            nc.vector.tensor_mul(qk[:, :nct_lo * P], k_lo, qv[:, :nct_lo * P])
            nc.vector.tensor_mul(qk[:, nct_lo * P:], k_hi, qv[:, nct_lo * P:])
            qksum = smp.tile([P, nct * H], f32, tag="qksum")
            nc.vector.tensor_reduce(
                qksum[:], qk[:].rearrange("p (a b) -> p a b", b=D),
                axis=mybir.AxisListType.X, op=mybir.AluOpType.add,
            )

            # edge projection (+ per-head sums) and gates, per chunk
            gate = gp.tile([P, nct * P], f32, tag="gate")
            logits = smp.tile([P, nct * H], f32, tag="logits")
            for c in range(nct):
                e_ps = eps_pool.tile([P, HD + H], f32, tag="eps")
                nc.tensor.matmul(
                    e_ps[:], lhsT=eaT[:, c * P:(c + 1) * P], rhs=We_sb[:],
                    start=True, stop=True,
                )
                nc.scalar.activation(
                    gate[:, c * P:(c + 1) * P], e_ps[:, :HD],
                    mybir.ActivationFunctionType.Sigmoid,
                )
                nc.vector.tensor_add(
                    logits[:, c * H:(c + 1) * H],
                    qksum[:, c * H:(c + 1) * H], e_ps[:, HD:],
                )

            # leaky_relu(logits * 0.25, alpha=0.2) then exp
            lr_a = smp.tile([P, nct * H], f32, tag="lr_a")
            nc.vector.tensor_scalar_mul(lr_a[:], logits[:], 0.25)
            lr_b = smp.tile([P, nct * H], f32, tag="lr_b")
            nc.vector.tensor_scalar_mul(lr_b[:], logits[:], 0.05)
            nc.vector.tensor_max(lr_a[:], lr_a[:], lr_b[:])
            ex = smp.tile([P, nct * H], f32, tag="ex")
            nc.scalar.activation(ex[:], lr_a[:], mybir.ActivationFunctionType.Exp)

            # rhs for the accumulation matmul: [msg (128 cols) | ex (8 cols)]
            rhs = rhp.tile([P, nct * (HD + H)], msg_dt, tag="rhs")
            rstride = HD + H
            kg = qkp.tile([P, nct * P], f32, tag="kg")
            nc.vector.tensor_mul(kg[:, :nct_lo * P], k_lo, gate[:, :nct_lo * P])
            nc.vector.tensor_mul(kg[:, nct_lo * P:], k_hi, gate[:, nct_lo * P:])
            pst = rhs[:].ap[0][0]
            msg_out = strided(rhs[:], [[pst, P], [rstride, nct], [D, H], [1, D]])
            exs = ex[:].ap[0][0]
            ex4 = strided(ex[:], [[exs, P], [H, nct], [1, H], [0, D]])
            nc.vector.tensor_mul(
                msg_out,
                kg[:].rearrange("p (c h d) -> p c h d", h=H, d=D),
                ex4,
            )
            ex_out = strided(rhs[:], [[pst, P], [rstride, nct], [1, H]],
                             extra_off=HD)
            nc.vector.tensor_copy(ex_out, ex[:].rearrange("p (c h) -> p c h", h=H))

            # accumulate [feat | den] over chunks in PSUM
            acc = accp.tile([P, HD + H], f32, tag="acc")
            for c in range(nct):
                nc.tensor.matmul(
                    acc[:],
                    lhsT=onehot[:, c * P:(c + 1) * P],
                    rhs=rhs[:, c * rstride:(c + 1) * rstride],
                    start=(c == 0), stop=(c == nct - 1),
                )

            # out = feat / (den + 1e-9)
            dplus = outp.tile([P, H], f32, tag="dplus")
            nc.vector.tensor_scalar_add(dplus[:], acc[:, HD:], 1e-9)
            recip = outp.tile([P, H], f32, tag="recip")
            nc.vector.reciprocal(recip[:], dplus[:])
            out_sb = outp.tile([P, HD], f32, tag="out_sb")
            rst = recip[:].ap[0][0]
            nc.vector.tensor_mul(
                out_sb[:].rearrange("p (h d) -> p h d", d=D),
                acc[:, :HD].rearrange("p (h d) -> p h d", d=D),
                strided(recip[:], [[rst, P], [1, H], [0, D]]),
            )
            nc.sync.dma_start(out_t.ap()[t * P:(t + 1) * P, :], out_sb[:])

    nc.compile()
    return nc


# --------------------------------------------------------------------------
# entry point
# --------------------------------------------------------------------------
def _chunk_counts(src, dst):
    """Max lo/hi chunk counts over (core, tile) given per-core rotation."""
    src = np.ascontiguousarray(src).astype(np.int64)
    dst = np.ascontiguousarray(dst).astype(np.int64)
    order = np.argsort(dst, kind="stable")
    src_s, dst_s = src[order], dst[order]
    bounds = np.searchsorted(dst_s // P, np.arange(NT_G + 1))
    nct_lo = nct_hi = 0
    for t in range(NT_G):
        s = src_s[bounds[t]:bounds[t + 1]]
        if len(s) == 0:
            continue
        base = (t // NT_PER_CORE) * NODES_PER_CORE
        rot = (s - base) % H_ROWS
        n_lo = int((rot < H_HALF).sum())
        n_hi = len(s) - n_lo
        nct_lo = max(nct_lo, -(-n_lo // P))
        nct_hi = max(nct_hi, -(-n_hi // P))
    return max(nct_lo, NCT_LO_DEFAULT), max(nct_hi, NCT_HI_DEFAULT)


def kernel(x, edge_attr, W_node, W_edge, src, dst):
    global _last_results
    from concourse.bass_utils import run_bass_kernel_spmd

    out_dtype = np.asarray(x).dtype
    nct_lo, nct_hi = _chunk_counts(src, dst)
    in_maps = _prep(x, edge_attr, W_node, W_edge, src, dst, nct_lo, nct_hi)
    nc = build_program(nct_lo, nct_hi)

    res = run_bass_kernel_spmd(
        nc, in_maps, core_ids=list(range(N_CORES)),
        trace=bool(int(os.environ.get("KERNEL_TRACE", "0"))),
    )
    _last_results = res
    if res.exec_time_ns is not None:
        print(f"HW exec time: {res.exec_time_ns} ns")

    out = np.concatenate([res.results[c]["out"] for c in range(N_CORES)], axis=0)
    return np.ascontiguousarray(out[:N_NODES]).astype(out_dtype)


# revision 10
# speedup vs baseline: 1.2129x; 1.2129x over previous
"""EG_GATLayer (multi-head edge-gated GAT) on 8 Trainium2 NeuronCores.

Strategy: sort edges by destination node on the host and assign each core a
contiguous range of 49 destination-node tiles (128 nodes each).  Each core
fully owns its output rows, so there are no collectives.  Node features h are
computed (replicated) on every core into a per-core ROTATED table (row r holds
node (r + core_base) mod 51200, achieved by rotating xT on the host), so
destination indices are local [0, 6272) and source indices split into two
int16-addressable half-tables for dma_gather.  Segment softmax + scatter-sum
are done with one-hot matmuls accumulating in PSUM; normalization by the
softmax denominator happens after aggregation (algebraically identical).
"""

import os
import sys

import numpy as np

for _p in ("/opt/trn_rl_repo", "/opt/pypackages"):
    if _p not in sys.path:
        sys.path.insert(0, _p)

import ml_dtypes

# ---- problem constants (hardcoded per contract) ----
N_NODES = 50000
N_EDGES = 800000
IN_DIM = 128
EDGE_DIM = 16
H = 8
D = 16
HD = H * D  # 128

P = 128
N_CORES = 8
NT_PER_CORE = 49
NT_G = NT_PER_CORE * N_CORES  # 392 tiles of 128 nodes >= 50000
NODES_PER_CORE = NT_PER_CORE * P  # 6272
H_BLOCKS = 25
H_ROWS = H_BLOCKS * 2048  # 51200 padded rows of h / cols of xT
H_HALF = H_ROWS // 2  # 25600 (int16-addressable half-table split)

NCT_LO_DEFAULT = 9
NCT_HI_DEFAULT = 9

# precision switches
MSG_BF16 = True  # one-hot + message/denominator accumulation matmuls in bf16
E_BF16 = True  # edge projection matmul in bf16
H_BF16 = True  # h table (gathered k/q rows) in bf16

_last_results = None  # test.py introspection


def _wrap_idx(idx_list):
    """dma_gather index layout: [16, n/16] wrapped, replicated to 128 parts."""
    n = len(idx_list)
    assert n % 16 == 0
    w = idx_list.reshape(n // 16, 16).T  # [16, n/16]
    return np.tile(w, (8, 1)).astype(np.int16)


# --------------------------------------------------------------------------
# host-side preprocessing
# --------------------------------------------------------------------------
def _prep(x, edge_attr, W_node, W_edge, src, dst, nct_lo, nct_hi):
    f32 = np.float32
    bf16 = ml_dtypes.bfloat16
    nct = nct_lo + nct_hi
    src = np.ascontiguousarray(src).astype(np.int64)
    dst = np.ascontiguousarray(dst).astype(np.int64)

    order = np.argsort(dst, kind="stable")
    src_s = src[order]
    dst_s = dst[order]
    ea_s = np.ascontiguousarray(edge_attr[order]).astype(f32)

    tile_id = dst_s // P
    bounds = np.searchsorted(tile_id, np.arange(NT_G + 1))

    e_dt = bf16 if E_BF16 else f32
    # gather idx arrays, per tile: [klo+q (nct_lo+nct)*128 idxs | khi nct_hi*128]
    n_t0 = (nct_lo + nct) * P
    n_t1 = nct_hi * P
    idx0_all = np.zeros((N_CORES, NT_PER_CORE, P, n_t0 // 16), np.int16)
    idx1_all = np.zeros((N_CORES, NT_PER_CORE, P, n_t1 // 16), np.int16)
    dstloc_all = np.full((N_CORES, NT_PER_CORE, P, nct), -1.0, bf16)
    eaT_all = np.zeros((N_CORES, NT_PER_CORE, 16, nct * P), e_dt)

    xT_pad = np.zeros((P, H_ROWS), f32)
    xT_pad[:, :N_NODES] = np.ascontiguousarray(x.astype(f32).T)

    in_maps = []
    xTs = []
    for c in range(N_CORES):
        base = c * NODES_PER_CORE
        xTs.append(np.ascontiguousarray(np.roll(xT_pad, -base, axis=1)))

    for t in range(NT_G):
        lo_b, hi_b = bounds[t], bounds[t + 1]
        cnt = hi_b - lo_b
        if cnt == 0:
            continue
        c = t // NT_PER_CORE
        j = t % NT_PER_CORE
        base = c * NODES_PER_CORE

        rot = (src_s[lo_b:hi_b] - base) % H_ROWS
        is_lo = rot < H_HALF
        n_lo = int(is_lo.sum())
        n_hi = cnt - n_lo
        assert n_lo <= nct_lo * P and n_hi <= nct_hi * P, (t, n_lo, n_hi)

        # slot assignment: lo edges fill chunks [0, nct_lo), hi fill the rest
        perm = np.concatenate([np.where(is_lo)[0], np.where(~is_lo)[0]])
        slot = np.empty(cnt, np.int64)
        slot[:n_lo] = np.arange(n_lo)
        slot[n_lo:] = nct_lo * P + np.arange(n_hi)
        rot_p = rot[perm]
        dst_p = dst_s[lo_b:hi_b][perm]
        ea_p = ea_s[lo_b:hi_b][perm]

        cc, pp = slot // P, slot % P
        dstloc_all[c, j, pp, cc] = (dst_p - t * P).astype(bf16)
        eaT_all[c, j, :, cc * P + pp] = ea_p.astype(e_dt)

        # table-0 gather: k-lo (chunks 0..nct_lo) then q (all nct chunks)
        i0 = np.zeros(n_t0, np.int64)
        i0[slot[:n_lo]] = rot_p[:n_lo]
        qi = np.zeros(nct * P, np.int64)
        qi[slot] = dst_p - base
        i0[nct_lo * P:] = qi
        idx0_all[c, j] = _wrap_idx(i0)
        # table-1 gather: k-hi (chunks nct_lo..nct)
        i1 = np.zeros(n_t1, np.int64)
        i1[slot[n_lo:] - nct_lo * P] = rot_p[n_lo:] - H_HALF
        idx1_all[c, j] = _wrap_idx(i1)

    Wn_cat = np.ascontiguousarray(
        W_node.astype(f32).transpose(1, 0, 2).reshape(IN_DIM, HD)
    )
    We_cat = np.zeros((EDGE_DIM, HD + H), f32)
    We_cat[:, :HD] = W_edge.astype(f32).transpose(1, 0, 2).reshape(EDGE_DIM, HD)
    We_cat[:, HD:] = W_edge.astype(f32).sum(axis=2).T
    We_cat = We_cat.astype(e_dt)

    iota = np.broadcast_to(np.arange(P), (P, P)).astype(bf16)

    for c in range(N_CORES):
        in_maps.append({
            "xT": xTs[c],
            "Wn": Wn_cat,
            "We": We_cat,
            "iota": iota,
            "idx0": np.ascontiguousarray(idx0_all[c]),
            "idx1": np.ascontiguousarray(idx1_all[c]),
            "dstloc": np.ascontiguousarray(dstloc_all[c]),
            "eaT": np.ascontiguousarray(eaT_all[c]),
        })
    return in_maps


# --------------------------------------------------------------------------
# device program
# --------------------------------------------------------------------------
def build_program(nct_lo, nct_hi):
    import concourse.bass as bass
    import concourse.mybir as mybir
    import concourse.tile as tile
    from concourse import bacc
    from contextlib import ExitStack

    f32 = mybir.dt.float32
    i16 = mybir.dt.int16
    bf16 = mybir.dt.bfloat16
    msg_dt = bf16 if MSG_BF16 else f32
    e_dt = bf16 if E_BF16 else f32
    h_dt = bf16 if H_BF16 else f32
    nct = nct_lo + nct_hi
    assert nct % 3 == 0
    n_t0 = (nct_lo + nct) * P
    n_t1 = nct_hi * P

    nc = bacc.Bacc("TRN2", target_bir_lowering=False, debug=False,
                   num_devices=N_CORES, num_swdge_queues=4)

    xT_t = nc.dram_tensor("xT", [P, H_ROWS], f32, kind="ExternalInput")
    Wn_t = nc.dram_tensor("Wn", [IN_DIM, HD], f32, kind="ExternalInput")
    We_t = nc.dram_tensor("We", [EDGE_DIM, HD + H], e_dt, kind="ExternalInput")
    iota_t = nc.dram_tensor("iota", [P, P], bf16, kind="ExternalInput")
    idx0_t = nc.dram_tensor("idx0", [NT_PER_CORE, P, n_t0 // 16], i16,
                            kind="ExternalInput")
    idx1_t = nc.dram_tensor("idx1", [NT_PER_CORE, P, n_t1 // 16], i16,
                            kind="ExternalInput")
    dstloc_t = nc.dram_tensor("dstloc", [NT_PER_CORE, P, nct], bf16,
                              kind="ExternalInput")
    eaT_t = nc.dram_tensor("eaT", [NT_PER_CORE, 16, nct * P], e_dt,
                           kind="ExternalInput")
    out_t = nc.dram_tensor("out", [NODES_PER_CORE, HD], f32,
                           kind="ExternalOutput")
    h_t = nc.dram_tensor("h", [H_ROWS, HD], h_dt)  # internal scratch (rotated)

    def strided(ap, pattern, extra_off=0):
        return bass.AP(ap.tensor, ap.offset + extra_off, pattern)

    with tile.TileContext(nc) as tc, ExitStack() as ctx:
        const = ctx.enter_context(tc.tile_pool(name="const", bufs=1))
        xbp = ctx.enter_context(tc.tile_pool(name="xb", bufs=3))
        hbp = ctx.enter_context(tc.tile_pool(name="hb", bufs=3))
        hps = ctx.enter_context(tc.tile_pool(name="hps", bufs=2, space="PSUM"))
        idxp = ctx.enter_context(tc.tile_pool(name="idx", bufs=2))
        kqp = ctx.enter_context(tc.tile_pool(name="kq", bufs=2))
        ohp = ctx.enter_context(tc.tile_pool(name="oh", bufs=2))
        qkp = ctx.enter_context(tc.tile_pool(name="qk", bufs=2))
        smp = ctx.enter_context(tc.tile_pool(name="sm", bufs=2))
        gp = ctx.enter_context(tc.tile_pool(name="g", bufs=2))
        rhp = ctx.enter_context(tc.tile_pool(name="rh", bufs=2))
        outp = ctx.enter_context(tc.tile_pool(name="o", bufs=2))
        eps_pool = ctx.enter_context(tc.tile_pool(name="eps", bufs=2, space="PSUM"))
        accp = ctx.enter_context(tc.tile_pool(name="acc", bufs=2, space="PSUM"))

        Wn_sb = const.tile([IN_DIM, HD], f32)
        nc.sync.dma_start(Wn_sb[:], Wn_t.ap())
        We_sb = const.tile([EDGE_DIM, HD + H], e_dt)
        nc.sync.dma_start(We_sb[:], We_t.ap())
        iota_sb = const.tile([P, P], bf16)
        nc.sync.dma_start(iota_sb[:], iota_t.ap())

        # ---- phase 0: h = x @ Wn for all nodes, row-major (rotated) ----
        for blk in range(H_BLOCKS):
            xb = xbp.tile([P, 2048], f32, tag="xb")
            nc.sync.dma_start(xb[:], xT_t.ap()[:, blk * 2048:(blk + 1) * 2048])
            hb = hbp.tile([P, 2048], h_dt, tag="hb")
            for g in range(4):
                hp = hps.tile([P, 512], f32, tag="hp")
                for s_ in range(4):
                    nc.tensor.matmul(
                        hp[:, s_ * P:(s_ + 1) * P],
                        lhsT=xb[:, (g * 4 + s_) * P:(g * 4 + s_ + 1) * P],
                        rhs=Wn_sb[:],
                        start=True, stop=True,
                    )
                nc.vector.tensor_copy(hb[:, g * 512:(g + 1) * 512], hp[:])
            # rows blk*2048 + (g*128 + p), iterate (p, g, f)
            h_out = strided(
                h_t.ap(), [[HD, P], [P * HD, 16], [1, HD]],
                extra_off=blk * 2048 * HD,
            )
            nc.sync.dma_start(h_out, hb[:].rearrange("p (g f) -> p g f", f=P))

        h_lo = h_t.ap()[:H_HALF, :]
        h_hi = h_t.ap()[H_HALF:, :]

        prev_gather = [None]

        def chain(inst):
            if prev_gather[0] is not None:
                tile.add_dep_helper(inst.ins, prev_gather[0].ins, False,
                                    reason="swdge queue/lane order")
            prev_gather[0] = inst

        # ---- phase 1: per destination-node tile ----
        for t in range(NT_PER_CORE):
            idx0 = idxp.tile([P, n_t0 // 16], i16, tag="idx0")
            nc.sync.dma_start(idx0[:], idx0_t.ap()[t])
            idx1 = idxp.tile([P, n_t1 // 16], i16, tag="idx1")
            nc.sync.dma_start(idx1[:], idx1_t.ap()[t])
            dstloc = idxp.tile([P, nct], bf16, tag="dstloc")
            nc.sync.dma_start(dstloc[:], dstloc_t.ap()[t])
            eaT = idxp.tile([16, nct * P], e_dt, tag="eaT")
            nc.sync.dma_start(eaT[:], eaT_t.ap()[t])

            # gathered rows, layout [k-lo (nct_lo) | q (nct) | k-hi (nct_hi)]
            kq = kqp.tile([P, (2 * nct) * P], h_dt, tag="kq")
            chain(nc.gpsimd.dma_gather(
                out_ap=kq[:, :n_t0].rearrange("p (c f) -> p c f", f=P),
                in_ap=h_lo,
                idxs_ap=idx0[:],
                num_idxs=n_t0,
                num_idxs_reg=n_t0,
                elem_size=HD,
                single_packet=False,
                queue_num=(2 * t) % 4,
            ))
            chain(nc.gpsimd.dma_gather(
                out_ap=kq[:, n_t0:].rearrange("p (c f) -> p c f", f=P),
                in_ap=h_hi,
                idxs_ap=idx1[:],
                num_idxs=n_t1,
                num_idxs_reg=n_t1,
                elem_size=HD,
                single_packet=False,
                queue_num=(2 * t + 1) % 4,
            ))
            k_lo = kq[:, :nct_lo * P]
            qv = kq[:, nct_lo * P:(nct_lo + nct) * P]
            k_hi = kq[:, (nct_lo + nct) * P:]

            # one-hot [edge, local-node] (pad edges have dstloc=-1 -> all 0)
            onehot = ohp.tile([P, nct * P], msg_dt, tag="onehot")
            oh_ap = onehot[:].rearrange("p (c n) -> p c n", n=P)
            d_ap = strided(dstloc[:], [[dstloc[:].ap[0][0], P], [1, nct], [0, P]])
            i_ap = strided(iota_sb[:], [[iota_sb[:].ap[0][0], P], [0, nct], [1, P]])
            nc.vector.tensor_tensor(oh_ap, d_ap, i_ap, op=mybir.AluOpType.is_equal)

            # qk dot per head (k in two pieces)
            qk = qkp.tile([P, nct * P], f32, tag="qk")
            nc.vector.tensor_mul(qk[:, :nct_lo * P], k_lo, qv[:, :nct_lo * P])
            nc.vector.tensor_mul(qk[:, nct_lo * P:], k_hi, qv[:, nct_lo * P:])
            qksum = smp.tile([P, nct * H], f32, tag="qksum")
            nc.vector.tensor_reduce(
                qksum[:], qk[:].rearrange("p (a b) -> p a b", b=D),
                axis=mybir.AxisListType.X, op=mybir.AluOpType.add,
            )

            # edge projection (+ per-head sums) and gates, 3 chunks per bank
            gate = gp.tile([P, nct * P], msg_dt, tag="gate")
            logits = smp.tile([P, nct * H], f32, tag="logits")
            for c3 in range(nct // 3):
                e_ps = eps_pool.tile([P, 3 * (HD + H)], f32, tag="eps")
                for cc in range(3):
                    c = c3 * 3 + cc
                    nc.tensor.matmul(
                        e_ps[:, cc * (HD + H):(cc + 1) * (HD + H)],
                        lhsT=eaT[:, c * P:(c + 1) * P], rhs=We_sb[:],
                        start=True, stop=True,
                    )
                pse = e_ps[:].ap[0][0]
                nc.scalar.activation(
                    gate[:, c3 * 3 * P:(c3 + 1) * 3 * P]
                        .rearrange("p (c f) -> p c f", f=P),
                    strided(e_ps[:], [[pse, P], [HD + H, 3], [1, HD]]),
                    mybir.ActivationFunctionType.Sigmoid,
                )
                nc.vector.tensor_add(
                    logits[:, c3 * 3 * H:(c3 + 1) * 3 * H]
                        .rearrange("p (c h) -> p c h", h=H),
                    qksum[:, c3 * 3 * H:(c3 + 1) * 3 * H]
                        .rearrange("p (c h) -> p c h", h=H),
                    strided(e_ps[:], [[pse, P], [HD + H, 3], [1, H]],
                            extra_off=HD),
                )

            # leaky_relu(logits * 0.25, alpha=0.2) then exp -> rhs_den (= ex)
            lr_a = smp.tile([P, nct * H], f32, tag="lr_a")
            nc.vector.tensor_scalar_mul(lr_a[:], logits[:], 0.25)
            lr_b = smp.tile([P, nct * H], f32, tag="lr_b")
            nc.vector.tensor_scalar_mul(lr_b[:], logits[:], 0.05)
            lr_m = smp.tile([P, nct * H], f32, tag="lr_m")
            nc.vector.tensor_max(lr_m[:], lr_a[:], lr_b[:])
            rhs_den = rhp.tile([P, nct * H], msg_dt, tag="rhs_den")
            nc.scalar.activation(rhs_den[:], lr_m[:],
                                 mybir.ActivationFunctionType.Exp)

            # msg = k * gate * ex
            rhs_msg = rhp.tile([P, nct * HD], msg_dt, tag="rhs_msg")
            kg = qkp.tile([P, nct * P], msg_dt, tag="kg")
            nc.vector.tensor_mul(kg[:, :nct_lo * P], k_lo, gate[:, :nct_lo * P])
            nc.vector.tensor_mul(kg[:, nct_lo * P:], k_hi, gate[:, nct_lo * P:])
            des = rhs_den[:].ap[0][0]
            ex4 = strided(rhs_den[:], [[des, P], [H, nct], [1, H], [0, D]])
            nc.vector.tensor_mul(
                rhs_msg[:].rearrange("p (c h d) -> p c h d", h=H, d=D),
                kg[:].rearrange("p (c h d) -> p c h d", h=H, d=D),
                ex4,
            )

            # accumulate feat and den over chunks in PSUM
            acc_f = accp.tile([P, HD], f32, tag="acc_f")
            acc_d = accp.tile([P, H], f32, tag="acc_d")
            for c in range(nct):
                nc.tensor.matmul(
                    acc_f[:],
                    lhsT=onehot[:, c * P:(c + 1) * P],
                    rhs=rhs_msg[:, c * HD:(c + 1) * HD],
                    start=(c == 0), stop=(c == nct - 1),
                )
                nc.tensor.matmul(
                    acc_d[:],
                    lhsT=onehot[:, c * P:(c + 1) * P],
                    rhs=rhs_den[:, c * H:(c + 1) * H],
                    start=(c == 0), stop=(c == nct - 1),
                )

            # out = feat / (den + 1e-9)
            dplus = outp.tile([P, H], f32, tag="dplus")
            nc.vector.tensor_scalar_add(dplus[:], acc_d[:], 1e-9)
            recip = outp.tile([P, H], f32, tag="recip")
            nc.vector.reciprocal(recip[:], dplus[:])
            out_sb = outp.tile([P, HD], f32, tag="out_sb")
            rst = recip[:].ap[0][0]
            nc.vector.tensor_mul(
                out_sb[:].rearrange("p (h d) -> p h d", d=D),
                acc_f[:].rearrange("p (h d) -> p h d", d=D),
                strided(recip[:], [[rst, P], [1, H], [0, D]]),
            )
            nc.sync.dma_start(out_t.ap()[t * P:(t + 1) * P, :], out_sb[:])

    nc.compile()
    return nc


# --------------------------------------------------------------------------
# entry point
# --------------------------------------------------------------------------
def _chunk_counts(src, dst):
    """Max lo/hi chunk counts over (core, tile) given per-core rotation."""
    src = np.ascontiguousarray(src).astype(np.int64)
    dst = np.ascontiguousarray(dst).astype(np.int64)
    order = np.argsort(dst, kind="stable")
    src_s, dst_s = src[order], dst[order]
    bounds = np.searchsorted(dst_s // P, np.arange(NT_G + 1))
    nct_lo = nct_hi = 0
    for t in range(NT_G):
        s = src_s[bounds[t]:bounds[t + 1]]
        if len(s) == 0:
            continue
        base = (t // NT_PER_CORE) * NODES_PER_CORE
        rot = (s - base) % H_ROWS
        n_lo = int((rot < H_HALF).sum())
        n_hi = len(s) - n_lo
        nct_lo = max(nct_lo, -(-n_lo // P))
        nct_hi = max(nct_hi, -(-n_hi // P))
    return max(nct_lo, NCT_LO_DEFAULT), max(nct_hi, NCT_HI_DEFAULT)


def kernel(x, edge_attr, W_node, W_edge, src, dst):
    global _last_results
    from concourse.bass_utils import run_bass_kernel_spmd

    out_dtype = np.asarray(x).dtype
    nct_lo, nct_hi = _chunk_counts(src, dst)
    in_maps = _prep(x, edge_attr, W_node, W_edge, src, dst, nct_lo, nct_hi)
    nc = build_program(nct_lo, nct_hi)

    res = run_bass_kernel_spmd(
        nc, in_maps, core_ids=list(range(N_CORES)),
        trace=bool(int(os.environ.get("KERNEL_TRACE", "0"))),
    )
    _last_results = res
    if res.exec_time_ns is not None:
        print(f"HW exec time: {res.exec_time_ns} ns")

    out = np.concatenate([res.results[c]["out"] for c in range(N_CORES)], axis=0)
    return np.ascontiguousarray(out[:N_NODES]).astype(out_dtype)
